# revision 9
# baseline (speedup 1.0000x reference)
"""Trainium2 Bass kernel for ComplexProjection:
    out[b,r,p] = |sum_s complex(x_real,x_imag)[b,r,s] * projection[r,s,p]|

Strategy: data-parallel over the particle axis B across 8 NeuronCores.
HBM traffic and PSUM-evacuation engine-time are the binding resources,
so (tolerance is 2e-2):

  x shipped as [r, s, {re, im}, b] fp8 e3m4     (16.8 MB per core)
  w as [s, r, p] fp16 (mixed-dtype matmul)      (0.5 MB)
  device computes ssq = re^2 + im^2, stores bf16 [r, p, b] (16.8 MB);
  the host takes the sqrt.  Measured end-to-end rel err ~9.6e-3.

Layout trick: per 1024-wide b-group, re and im land in the two halves
of ONE [P, 2048] PSUM tile (4 banks; 4 matmuls at N=512).  The whole
group is then evacuated+squared by a single FD=2048 op, and the two
halves summed by one FD=1024 DVE bf16 add (2x mode, distinct ports):
    ACT:  sq = ps^2 -> bf16 [P,2048]     (most groups)
    DVE:  cp = cast(ps); GP: sq = cp*cp  (some groups, to balance)
    DVE/GP: out[:,g] = sq[:,:1024] + sq[:,1024:]
fp32 PSUM reads are 1x everywhere (one PSUM port), which is why the
evacuation is the scarce resource; bf16 SBUF tensor_tensor runs 2x on
DVE but NOT with the same operand twice (same-address port conflict),
hence squares happen on ACT (fused) or GPSIMD.
"""

import os

import numpy as np

B, R, S, P = 32768, 16, 128, 128
NCORES = 8
BC = B // NCORES  # 4096 particles per core
GRP = 1024        # out columns per PSUM group ([P, 2*GRP] fp32 = 4 banks)
MMN = 512         # matmul moving dim (one bank)

XSUB = int(os.environ.get("KXSUB", "4096"))  # b-range per x DMA
NXS = BC // XSUB

XDT = os.environ.get("KXDT", "fp8e3")  # fp8e3 | fp16
WDT = os.environ.get("KWDT", "fp16")   # fp16 | fp8e3 (scaled by 16)

# per group-index mod 8: (evac engine, add engine).  "A" = ACT fused
# square; "V" = DVE cast + GPSIMD square.
ROT = [
    ("A", "V"), ("A", "V"), ("A", "V"), ("V", "G"),
    ("A", "V"), ("A", "V"), ("A", "V"), ("V", "V"),
]

_prog_cache = {}


def _build(nc, tile, mybir):
    f32 = mybir.dt.float32
    f16 = mybir.dt.float16
    bf16 = mybir.dt.bfloat16
    xdt = {"fp8e3": mybir.dt.float8e3, "fp16": f16}[XDT]
    wdt = {"fp8e3": mybir.dt.float8e3, "fp16": f16}[WDT]
    x = nc.dram_tensor("x", [R, S, 2, BC], xdt, kind="ExternalInput")
    w = nc.dram_tensor("w", [S, R, P], wdt, kind="ExternalInput")
    o = nc.dram_tensor("o", [R, P, BC], bf16, kind="ExternalOutput")
    x_ap, w_ap, o_ap = x.ap(), w.ap(), o.ap()

    with tile.TileContext(nc) as tc:
        with (
            tc.tile_pool(name="wp", bufs=1) as wp,
            tc.tile_pool(name="xp", bufs=int(os.environ.get("KXBUFS", "4"))) as xp,
            tc.tile_pool(name="op", bufs=int(os.environ.get("KOBUFS", "4"))) as op,
            tc.tile_pool(name="sq", bufs=4) as sqp,
            tc.tile_pool(name="ps", bufs=2, space="PSUM") as psp,
        ):
            w_sb = wp.tile([S, R, P], wdt, tag="w")
            nc.sync.dma_start(w_sb[:], w_ap[:])

            gi = 0
            for r in range(R):
                wr = w_sb[:, r, :]
                for xs in range(NXS):
                    bsl = slice(xs * XSUB, (xs + 1) * XSUB)
                    x_sb = xp.tile([S, 2, XSUB], xdt, tag="x")
                    if r == 0 and xs == 0:
                        # split the very first slab so the first matmuls
                        # start as early as possible
                        q = XSUB // 4
                        for h in range(4):
                            nc.sync.dma_start(
                                x_sb[:, :, h * q:(h + 1) * q],
                                x_ap[r, :, :, h * q:(h + 1) * q])
                    else:
                        nc.sync.dma_start(x_sb[:], x_ap[r, :, :, bsl])
                    out_sb = op.tile([P, XSUB], bf16, tag="o")
                    for g in range(XSUB // GRP):
                        sl = slice(g * GRP, (g + 1) * GRP)
                        evac_e, add_e = ROT[gi % len(ROT)]
                        gi += 1
                        # re in ps[:, :GRP], im in ps[:, GRP:]
                        ps = psp.tile([P, 2 * GRP], f32, tag="ps")
                        xin = x_sb[:, :, sl]
                        for m in range(GRP // MMN):
                            msl = slice(m * MMN, (m + 1) * MMN)
                            nc.tensor.matmul(ps[:, m * MMN:(m + 1) * MMN],
                                             wr, xin[:, 0, msl],
                                             start=True, stop=True)
                            nc.tensor.matmul(ps[:, GRP + m * MMN:
                                                GRP + (m + 1) * MMN],
                                             wr, xin[:, 1, msl],
                                             start=True, stop=True)
                        sq = sqp.tile([P, 2 * GRP], bf16, tag="sq")
                        if evac_e == "A":
                            nc.scalar.square(sq[:], ps[:])
                        else:
                            cp = sqp.tile([P, 2 * GRP], bf16, tag="cp")
                            nc.vector.tensor_copy(cp[:], ps[:])
                            nc.gpsimd.tensor_mul(sq[:], cp[:], cp[:])
                        eng = nc.vector if add_e == "V" else nc.gpsimd
                        eng.tensor_add(out_sb[:, sl], sq[:, :GRP], sq[:, GRP:])
                    if r == R - 1 and xs == NXS - 1:
                        # finer stores at the tail so the last compute
                        # overlaps its own writeback
                        h4 = XSUB // 4
                        for h in range(4):
                            nc.scalar.dma_start(
                                o_ap[r, :, xs * XSUB + h * h4:
                                     xs * XSUB + (h + 1) * h4],
                                out_sb[:, h * h4:(h + 1) * h4])
                    else:
                        nc.scalar.dma_start(o_ap[r, :, bsl], out_sb[:])


def _build_program():
    key = (XDT, WDT, XSUB)
    if key in _prog_cache:
        return _prog_cache[key]

    import concourse.tile as tile
    from concourse import bacc, mybir

    nc = bacc.Bacc("TRN2", target_bir_lowering=False, debug=False,
                   num_devices=NCORES)
    _build(nc, tile, mybir)
    nc.compile()
    _prog_cache[key] = nc
    return nc


LAST_RESULT = None


def kernel(x_real, x_imag, projection):
    global LAST_RESULT
    import ml_dtypes
    from concourse.bass_utils import run_bass_kernel_spmd

    nc = _build_program()

    xdt = {"fp8e3": ml_dtypes.float8_e3m4, "fp16": np.float16}[XDT]
    w32 = np.ascontiguousarray(
        np.asarray(projection, dtype=np.float32).transpose(1, 0, 2))
    if WDT == "fp16":
        w = w32.astype(np.float16)
        oscale = 1.0
    else:
        w = (w32 * 16.0).astype(ml_dtypes.float8_e3m4)
        oscale = 1.0 / 16.0

    # x: (B, R, S) re/im fp32 -> [R, S, 2, B], sliced per core on b
    xt = np.empty((R, S, 2, B), dtype=xdt)
    xt[:, :, 0, :] = np.asarray(x_real, dtype=np.float32).transpose(1, 2, 0)
    xt[:, :, 1, :] = np.asarray(x_imag, dtype=np.float32).transpose(1, 2, 0)

    in_maps = []
    for c in range(NCORES):
        sl = slice(c * BC, (c + 1) * BC)
        in_maps.append({"x": np.ascontiguousarray(xt[:, :, :, sl]), "w": w})

    res = run_bass_kernel_spmd(nc, in_maps, core_ids=list(range(NCORES)))
    LAST_RESULT = res
    out = np.empty((B, R, P), dtype=np.float32)
    for c in range(NCORES):
        ssq = res.results[c]["o"].astype(np.float32)  # [R, P, BC]
        out[c * BC:(c + 1) * BC] = oscale * np.sqrt(ssq).transpose(2, 0, 1)
    return out


# revision 10
# speedup vs baseline: 1.1267x; 1.1267x over previous
"""Trainium2 Bass kernel for ComplexProjection:
    out[b,r,p] = |sum_s complex(x_real,x_imag)[b,r,s] * projection[r,s,p]|

Strategy: data-parallel over the particle axis B across 8 NeuronCores.
The kernel is HBM-bandwidth and PSUM-evacuation bound, so inputs and
outputs move in reduced precision (tolerance is 2e-2):

  x shipped as [r, s, {re, im}, b] fp8 e3m4     (16.8 MB per core)
  w as [s, r, p] fp16 (mixed-dtype matmul)      (0.5 MB)
  device computes ssq = re^2 + im^2, stores fp16 [r, p, b] (16.8 MB);
  the host takes the sqrt.  Measured end-to-end rel err ~9.5e-3
  (dominated by the e3m4 x quantization; fp8 subnormals are honored
  by the PE, verified against a numpy simulation of the same path).

Per r and 1024-wide b-chunk (two fp32 PSUM banks, 2 matmuls per
component at N=512):
    ps_re[p,c] = sum_s w[r,s,p] * x[r,s,0,c]    (PE matmul, W stationary)
    ps_im[p,c] = sum_s w[r,s,p] * x[r,s,1,c]
epilogue (GPSIMD cannot read PSUM; fp32 PSUM reads run 1x on DVE/ACT),
rotated across chunks so ACT/DVE/GPSIMD land ~equal busy time:
    ACT:  sq_i = ps_im^2 -> fp16 always; also sq_r on 3/8 of chunks
    DVE:  cp_r = copy(ps_re) on 5/8; mult/add split with GPSIMD
The host takes the final sqrt on the fp16 ssq.
"""

import os

import numpy as np

B, R, S, P = 32768, 16, 128, 128
NCORES = 8
BC = B // NCORES  # 4096 particles per core
CH = 1024         # epilogue chunk (two fp32 PSUM banks)
MMN = 512         # matmul moving dim (one bank)

XSUB = int(os.environ.get("KXSUB", "4096"))  # b-range per x DMA
NXS = BC // XSUB

XDT = os.environ.get("KXDT", "fp8e3")  # fp8e3 | fp16
WDT = os.environ.get("KWDT", "fp16")   # fp16 | fp8e3 (scaled by 16)

# (who_squares_r, who_mults, who_adds) per chunk-index mod 8.
# "A" = ACT square (no copy/mult needed), else DVE copies and the
# listed engine does the fp16 square; last slot is the add engine.
ROT = [
    ("A", None, "V"),
    ("V", "V", "V"),
    ("V", "G", "V"),
    ("A", None, "G"),
    ("V", "V", "G"),
    ("V", "G", "V"),
    ("A", None, "G"),
    ("V", "V", "V"),
]

_prog_cache = {}


def _build(nc, tile, mybir):
    f32 = mybir.dt.float32
    f16 = mybir.dt.float16
    xdt = {"fp8e3": mybir.dt.float8e3, "fp16": f16}[XDT]
    wdt = {"fp8e3": mybir.dt.float8e3, "fp16": f16}[WDT]
    x = nc.dram_tensor("x", [R, S, 2, BC], xdt, kind="ExternalInput")
    w = nc.dram_tensor("w", [S, R, P], wdt, kind="ExternalInput")
    o = nc.dram_tensor("o", [R, P, BC], f16, kind="ExternalOutput")
    x_ap, w_ap, o_ap = x.ap(), w.ap(), o.ap()

    with tile.TileContext(nc) as tc:
        with (
            tc.tile_pool(name="wp", bufs=1) as wp,
            tc.tile_pool(name="xp", bufs=int(os.environ.get("KXBUFS", "4"))) as xp,
            tc.tile_pool(name="op", bufs=int(os.environ.get("KOBUFS", "4"))) as op,
            tc.tile_pool(name="sq", bufs=6) as sqp,
            tc.tile_pool(name="ps", bufs=2, space="PSUM") as psp,
        ):
            w_sb = wp.tile([S, R, P], wdt, tag="w")
            nc.sync.dma_start(w_sb[:], w_ap[:])

            ci = 0
            for r in range(R):
                wr = w_sb[:, r, :]
                for xs in range(NXS):
                    bsl = slice(xs * XSUB, (xs + 1) * XSUB)
                    x_sb = xp.tile([S, 2, XSUB], xdt, tag="x")
                    if r == 0 and xs == 0:
                        # split the very first slab so the first matmuls
                        # start as early as possible
                        q = XSUB // 4
                        for h in range(4):
                            nc.sync.dma_start(
                                x_sb[:, :, h * q:(h + 1) * q],
                                x_ap[r, :, :, h * q:(h + 1) * q])
                    else:
                        nc.sync.dma_start(x_sb[:], x_ap[r, :, :, bsl])
                    out_sb = op.tile([P, XSUB], f16, tag="o")
                    for cc in range(XSUB // CH):
                        sl = slice(cc * CH, (cc + 1) * CH)
                        sqr_e, mul_e, add_e = ROT[ci % len(ROT)]
                        ci += 1
                        ps_r = psp.tile([P, CH], f32, tag="psr")
                        ps_i = psp.tile([P, CH], f32, tag="psi")
                        for m in range(CH // MMN):
                            msl = slice(m * MMN, (m + 1) * MMN)
                            xin = x_sb[:, :, sl]
                            nc.tensor.matmul(ps_r[:, msl], wr, xin[:, 0, msl],
                                             start=True, stop=True)
                            nc.tensor.matmul(ps_i[:, msl], wr, xin[:, 1, msl],
                                             start=True, stop=True)
                        sq_i = sqp.tile([P, CH], f16, tag="sqi")
                        nc.scalar.square(sq_i[:], ps_i[:])
                        sq_r = sqp.tile([P, CH], f16, tag="sqr")
                        if sqr_e == "A":
                            nc.scalar.square(sq_r[:], ps_r[:])
                        else:
                            cp_r = sqp.tile([P, CH], f16, tag="cpr")
                            nc.vector.tensor_copy(cp_r[:], ps_r[:])
                            eng = nc.vector if mul_e == "V" else nc.gpsimd
                            eng.tensor_mul(sq_r[:], cp_r[:], cp_r[:])
                        eng = nc.vector if add_e == "V" else nc.gpsimd
                        eng.tensor_add(out_sb[:, sl], sq_r[:], sq_i[:])
                    if r == R - 1 and xs == NXS - 1:
                        # finer stores at the tail so the last compute
                        # overlaps its own writeback
                        h4 = XSUB // 4
                        for h in range(4):
                            nc.scalar.dma_start(
                                o_ap[r, :, xs * XSUB + h * h4:
                                     xs * XSUB + (h + 1) * h4],
                                out_sb[:, h * h4:(h + 1) * h4])
                    else:
                        nc.scalar.dma_start(o_ap[r, :, bsl], out_sb[:])


def _build_program():
    key = (XDT, WDT, XSUB)
    if key in _prog_cache:
        return _prog_cache[key]

    import concourse.tile as tile
    from concourse import bacc, mybir

    nc = bacc.Bacc("TRN2", target_bir_lowering=False, debug=False,
                   num_devices=NCORES)
    _build(nc, tile, mybir)
    nc.compile()
    _prog_cache[key] = nc
    return nc


LAST_RESULT = None


def kernel(x_real, x_imag, projection):
    global LAST_RESULT
    import ml_dtypes
    from concourse.bass_utils import run_bass_kernel_spmd

    nc = _build_program()

    xdt = {"fp8e3": ml_dtypes.float8_e3m4, "fp16": np.float16}[XDT]
    w32 = np.ascontiguousarray(
        np.asarray(projection, dtype=np.float32).transpose(1, 0, 2))
    if WDT == "fp16":
        w = w32.astype(np.float16)
        oscale = 1.0
    else:
        w = (w32 * 16.0).astype(ml_dtypes.float8_e3m4)
        oscale = 1.0 / 16.0

    # x: (B, R, S) re/im fp32 -> [R, S, 2, B], sliced per core on b
    xt = np.empty((R, S, 2, B), dtype=xdt)
    xt[:, :, 0, :] = np.asarray(x_real, dtype=np.float32).transpose(1, 2, 0)
    xt[:, :, 1, :] = np.asarray(x_imag, dtype=np.float32).transpose(1, 2, 0)

    in_maps = []
    for c in range(NCORES):
        sl = slice(c * BC, (c + 1) * BC)
        in_maps.append({"x": np.ascontiguousarray(xt[:, :, :, sl]), "w": w})

    res = run_bass_kernel_spmd(nc, in_maps, core_ids=list(range(NCORES)))
    LAST_RESULT = res
    out = np.empty((B, R, P), dtype=np.float32)
    for c in range(NCORES):
        ssq = res.results[c]["o"].astype(np.float32)  # [R, P, BC]
        out[c * BC:(c + 1) * BC] = oscale * np.sqrt(ssq).transpose(2, 0, 1)
    return out


# revision 11
# speedup vs baseline: 1.2093x; 1.0732x over previous
"""Trainium2 Bass kernel for ComplexProjection:
    out[b,r,p] = |sum_s complex(x_real,x_imag)[b,r,s] * projection[r,s,p]|

Strategy: data-parallel over the particle axis B across 8 NeuronCores.
The kernel is HBM-bandwidth and PSUM-evacuation bound, so inputs and
outputs move in reduced precision (tolerance is 2e-2):

  x shipped as [r, s, {re, im}, b] fp8 e3m4     (16.8 MB per core)
  w as [s, r, p] fp16 (mixed-dtype matmul)      (0.5 MB)
  device computes ssq = re^2 + im^2, stores fp16 [r, p, b] (16.8 MB);
  the host takes the sqrt.  Measured end-to-end rel err ~9.5e-3
  (dominated by the e3m4 x quantization; fp8 subnormals are honored
  by the PE, verified against a numpy simulation of the same path).

Per r and 1024-wide b-chunk (two fp32 PSUM banks, 2 matmuls per
component at N=512):
    ps_re[p,c] = sum_s w[r,s,p] * x[r,s,0,c]    (PE matmul, W stationary)
    ps_im[p,c] = sum_s w[r,s,p] * x[r,s,1,c]
epilogue (GPSIMD cannot read PSUM; fp32 PSUM reads run 1x on DVE/ACT),
rotated across chunks so ACT/DVE/GPSIMD land ~equal busy time:
    ACT:  sq_i = ps_im^2 -> fp16 always; also sq_r on 3/8 of chunks
    DVE:  cp_r = copy(ps_re) on 5/8; mult/add split with GPSIMD
The host takes the final sqrt on the fp16 ssq.
"""

import os

import numpy as np

B, R, S, P = 32768, 16, 128, 128
NCORES = 8
BC = B // NCORES  # 4096 particles per core
CH = 1024         # epilogue chunk (two fp32 PSUM banks)
MMN = 512         # matmul moving dim (one bank)

XSUB = int(os.environ.get("KXSUB", "4096"))  # b-range per x DMA
NXS = BC // XSUB

XDT = os.environ.get("KXDT", "fp8e3")  # fp8e3 | fp16
WDT = os.environ.get("KWDT", "fp16")   # fp16 | fp8e3 (scaled by 16)

# (who_squares_r, who_mults, who_adds) per chunk-index mod 8.
# "A" = ACT square (no copy/mult needed), else DVE copies and the
# listed engine does the fp16 square; last slot is the add engine.
ROT = [
    ("A", None, "G"), ("V", "G", "V"), ("A", None, "V"), ("V", "G", "V"),
    ("A", None, "V"), ("V", "G", "V"), ("A", None, "V"), ("V", "G", "V"),
    ("A", None, "G"), ("V", "G", "V"), ("A", None, "V"), ("V", "G", "V"),
    ("V", "G", "V"), ("A", None, "V"), ("V", "G", "V"), ("V", "G", "V"),
]

_prog_cache = {}


def _build(nc, tile, mybir):
    f32 = mybir.dt.float32
    f16 = mybir.dt.float16
    bf16 = mybir.dt.bfloat16
    xdt = {"fp8e3": mybir.dt.float8e3, "fp16": f16}[XDT]
    wdt = {"fp8e3": mybir.dt.float8e3, "fp16": f16}[WDT]
    x = nc.dram_tensor("x", [R, S, 2, BC], xdt, kind="ExternalInput")
    w = nc.dram_tensor("w", [S, R, P], wdt, kind="ExternalInput")
    o = nc.dram_tensor("o", [R, P, BC], bf16, kind="ExternalOutput")
    x_ap, w_ap, o_ap = x.ap(), w.ap(), o.ap()

    with tile.TileContext(nc) as tc:
        with (
            tc.tile_pool(name="wp", bufs=1) as wp,
            tc.tile_pool(name="xp", bufs=int(os.environ.get("KXBUFS", "4"))) as xp,
            tc.tile_pool(name="op", bufs=int(os.environ.get("KOBUFS", "4"))) as op,
            tc.tile_pool(name="sq", bufs=6) as sqp,
            tc.tile_pool(name="ps", bufs=2, space="PSUM") as psp,
        ):
            w_sb = wp.tile([S, R, P], wdt, tag="w")
            nc.sync.dma_start(w_sb[:], w_ap[:])

            ci = 0
            for r in range(R):
                wr = w_sb[:, r, :]
                for xs in range(NXS):
                    bsl = slice(xs * XSUB, (xs + 1) * XSUB)
                    x_sb = xp.tile([S, 2, XSUB], xdt, tag="x")
                    if r == 0 and xs == 0:
                        # split the very first slab so the first matmuls
                        # start as early as possible
                        q = XSUB // 8
                        for h in range(8):
                            nc.sync.dma_start(
                                x_sb[:, :, h * q:(h + 1) * q],
                                x_ap[r, :, :, h * q:(h + 1) * q])
                    else:
                        nc.sync.dma_start(x_sb[:], x_ap[r, :, :, bsl])
                    out_sb = op.tile([P, XSUB], bf16, tag="o")
                    for cc in range(XSUB // CH):
                        sl = slice(cc * CH, (cc + 1) * CH)
                        sqr_e, mul_e, add_e = ROT[ci % len(ROT)]
                        ci += 1
                        ps_r = psp.tile([P, CH], f32, tag="psr")
                        ps_i = psp.tile([P, CH], f32, tag="psi")
                        for m in range(CH // MMN):
                            msl = slice(m * MMN, (m + 1) * MMN)
                            xin = x_sb[:, :, sl]
                            nc.tensor.matmul(ps_r[:, msl], wr, xin[:, 0, msl],
                                             start=True, stop=True)
                            nc.tensor.matmul(ps_i[:, msl], wr, xin[:, 1, msl],
                                             start=True, stop=True)
                        sq_i = sqp.tile([P, CH], bf16, tag="sqi")
                        nc.scalar.square(sq_i[:], ps_i[:])
                        sq_r = sqp.tile([P, CH], bf16, tag="sqr")
                        if sqr_e == "A":
                            nc.scalar.square(sq_r[:], ps_r[:])
                        else:
                            cp_r = sqp.tile([P, CH], bf16, tag="cpr")
                            nc.vector.tensor_copy(cp_r[:], ps_r[:])
                            eng = nc.vector if mul_e == "V" else nc.gpsimd
                            eng.tensor_mul(sq_r[:], cp_r[:], cp_r[:])
                        eng = nc.vector if add_e == "V" else nc.gpsimd
                        eng.tensor_add(out_sb[:, sl], sq_r[:], sq_i[:])
                    if r == R - 1 and xs == NXS - 1:
                        # finer stores at the tail so the last compute
                        # overlaps its own writeback
                        h4 = XSUB // 4
                        for h in range(4):
                            nc.scalar.dma_start(
                                o_ap[r, :, xs * XSUB + h * h4:
                                     xs * XSUB + (h + 1) * h4],
                                out_sb[:, h * h4:(h + 1) * h4])
                    else:
                        nc.scalar.dma_start(o_ap[r, :, bsl], out_sb[:])


def _build_program():
    key = (XDT, WDT, XSUB)
    if key in _prog_cache:
        return _prog_cache[key]

    import concourse.tile as tile
    from concourse import bacc, mybir

    nc = bacc.Bacc("TRN2", target_bir_lowering=False, debug=False,
                   num_devices=NCORES)
    _build(nc, tile, mybir)
    nc.compile()
    _prog_cache[key] = nc
    return nc


LAST_RESULT = None


def kernel(x_real, x_imag, projection):
    global LAST_RESULT
    import ml_dtypes
    from concourse.bass_utils import run_bass_kernel_spmd

    nc = _build_program()

    xdt = {"fp8e3": ml_dtypes.float8_e3m4, "fp16": np.float16}[XDT]
    w32 = np.ascontiguousarray(
        np.asarray(projection, dtype=np.float32).transpose(1, 0, 2))
    if WDT == "fp16":
        w = w32.astype(np.float16)
        oscale = 1.0
    else:
        w = (w32 * 16.0).astype(ml_dtypes.float8_e3m4)
        oscale = 1.0 / 16.0

    # x: (B, R, S) re/im fp32 -> [R, S, 2, B], sliced per core on b
    xt = np.empty((R, S, 2, B), dtype=xdt)
    xt[:, :, 0, :] = np.asarray(x_real, dtype=np.float32).transpose(1, 2, 0)
    xt[:, :, 1, :] = np.asarray(x_imag, dtype=np.float32).transpose(1, 2, 0)

    in_maps = []
    for c in range(NCORES):
        sl = slice(c * BC, (c + 1) * BC)
        in_maps.append({"x": np.ascontiguousarray(xt[:, :, :, sl]), "w": w})

    res = run_bass_kernel_spmd(nc, in_maps, core_ids=list(range(NCORES)))
    LAST_RESULT = res
    out = np.empty((B, R, P), dtype=np.float32)
    for c in range(NCORES):
        ssq = res.results[c]["o"].astype(np.float32)  # [R, P, BC]
        out[c * BC:(c + 1) * BC] = oscale * np.sqrt(ssq).transpose(2, 0, 1)
    return out
